# revision 4
# baseline (speedup 1.0000x reference)
"""Causal self-attention on 8 trn2 NeuronCores.

Sharding: core c -> (batch b = c//2, head-group g = c%2); each head-group is
8 heads = 512 channels.  Per core: q/k/v projections restricted to the
group's 512 columns, causal attention in the transposed orientation
S^T = [tk, tq] (softmax denominators come free from a ones-column in V),
partial output projection through the group's 512 rows of Wo.  Host sums
the two partials per batch and adds (bv @ Wo + bo).

Performance structure:
- QK matmuls run in fp8e4m3 DoubleRow mode (0.5 cycles/row): each head has
  only 64 contraction channels, so slot0 carries k and slot1 carries zeros;
  the zero slot nullifies whatever the paired q stream reads.
- The attention inner loop is software-pipelined: the AV matmuls of tk-pair
  p are emitted after the QK matmuls of pair p+1, so the exp of pair p
  (scalar or vector engine) overlaps the PE work of pair p+1.
- exp is split across engines per head: half the heads use the scalar
  engine's ACT Exp, half use a Schraudolph fast-exp2 on the vector engine
  (tensor_scalar f32->int16 writing biased-exponent bits, consumed by the
  AV matmul through a bitcast-to-bf16 view).  Softmax normalization
  cancels most of the approximation error.
- Diagonal tk-tiles stream only their causally-needed columns (slot1 of a
  diagonal pair is right-aligned to its shorter width); the masked corner
  is cleaned by mask/mask2 multiplies as in the transposed-orientation
  formulation.
- Output partials are written in bf16 to halve the drain DMA.
"""

import numpy as np
import ml_dtypes

import concourse.bass as bass
import concourse.mybir as mybir
from concourse import bacc, tile
from concourse.bass_utils import run_bass_kernel_spmd

B, T, C, H = 4, 2048, 1024, 16
HD = C // H          # 64
G = 2                # head groups (cores per batch)
HG = H // G          # 8 heads per group
CG = C // G          # 512 channels per group
CGP = CG // 128      # 4 c_out tiles per group
P = 128
W = 512              # free-dim window (one PSUM bank of f32)
NW = T // W          # 4 windows
NTT = T // P         # 16 t tiles
NCI = C // P         # 8 c_in chunks
VS = HD + 1          # 65: v plus ones column

ESC = 1.0 / float(np.sqrt(HD))
LOG2E = 1.4426950408889634
C_SCH = 0.0          # Schraudolph bias constant (tuned by sim)
SCH_HEADS = frozenset({0, 1, 2, 3})   # heads whose exp runs on the DVE

_cached_nc = None


def _build():
    f32 = mybir.dt.float32
    bf16 = mybir.dt.bfloat16
    f8 = mybir.dt.float8e4
    i16 = mybir.dt.int16
    AF = mybir.ActivationFunctionType
    DR = mybir.MatmulPerfMode.DoubleRow
    nc = bacc.Bacc("TRN2", target_bir_lowering=False, debug=False, num_devices=8)

    xt_d = nc.dram_tensor("xt", [C, T], bf16, kind="ExternalInput")
    wq_d = nc.dram_tensor("wq", [P, CGP, NCI * P], bf16, kind="ExternalInput")
    wk_d = nc.dram_tensor("wk", [P, CGP, NCI * P], bf16, kind="ExternalInput")
    wv_d = nc.dram_tensor("wv", [P, NCI, CG], bf16, kind="ExternalInput")
    wo_d = nc.dram_tensor("wo", [CG, C], bf16, kind="ExternalInput")
    bq_d = nc.dram_tensor("bq", [P, CGP], f32, kind="ExternalInput")
    bk_d = nc.dram_tensor("bk", [P, CGP], f32, kind="ExternalInput")
    mask_d = nc.dram_tensor("mask", [P, P], f32, kind="ExternalInput")
    mask2_d = nc.dram_tensor("mask2", [P, 2 * P], f32, kind="ExternalInput")
    out_d = nc.dram_tensor("outp", [C, T], bf16, kind="ExternalOutput")

    mm = lambda out, lhsT, rhs, start, stop: nc.tensor.matmul(
        out, lhsT, rhs, start=start, stop=stop)

    with tile.TileContext(nc) as tc:
        with (
            tc.tile_pool(name="pers", bufs=1) as pers,
            tc.tile_pool(name="wchunk", bufs=1) as wpool,
            tc.tile_pool(name="xchunk", bufs=NCI) as xpool,
            tc.tile_pool(name="attn", bufs=1) as attn,
            tc.tile_pool(name="pt", bufs=4) as ptpool,
            tc.tile_pool(name="dn", bufs=4) as dnpool,
            tc.tile_pool(name="rb", bufs=4) as rbpool,
            tc.tile_pool(name="osb", bufs=3) as opool,
        ):
            # fp8 q^T with one zero pad j-block (garbage slot for DoubleRow)
            qT = pers.tile([P, CGP + 1, T], f8)
            # kz[p, j, slot, t]: slot0 = k (head (j,0) rows 0:64, head (j,1)
            # rows 64:128), slot1 = zeros (DoubleRow null slot)
            kz = pers.tile([P, CGP, 2, T], f8)
            vp = pers.tile([P, NTT, HG * VS], bf16)  # v rows + ones col/head
            wo_sb = pers.tile([P, CGP, C], bf16)
            mask_sb = pers.tile([P, P], f32)
            mask2_sb = pers.tile([P, 2 * P], f32)
            bq_sb = pers.tile([P, CGP], f32)
            bk_sb = pers.tile([P, CGP], f32)
            ones_sb = pers.tile([P, HG], f32)
            yT = attn.tile([P, CGP, T], bf16)

            # ---- DMA issue order tuned for startup ----
            def dma_w_block(dst, src_d, j):
                nc.sync.dma_start(out=dst[:, j, :, :],
                                  in_=src_d.ap()[:, j, :].rearrange(
                                      "p (c n) -> p c n", n=P))

            wh_q = wpool.tile([P, CGP, NCI, P], bf16)
            wh_k = wpool.tile([P, CGP, NCI, P], bf16)
            wh_v = wpool.tile([P, NCI, CG], bf16)
            xc = [xpool.tile([P, T], bf16, tag="xc", name=f"xc{i}")
                  for i in range(NCI)]

            dma_w_block(wh_q, wq_d, 0)
            dma_w_block(wh_k, wk_d, 0)
            nc.sync.dma_start(out=bq_sb, in_=bq_d.ap())
            nc.sync.dma_start(out=bk_sb, in_=bk_d.ap())
            for i in range(NCI):   # quarter 0 of all chunks
                nc.sync.dma_start(out=xc[i][:, 0:W],
                                  in_=xt_d.ap()[i * P:(i + 1) * P, 0:W])
            dma_w_block(wh_q, wq_d, 1)
            dma_w_block(wh_k, wk_d, 1)
            for i in range(NCI):   # wv per chunk
                nc.sync.dma_start(out=wh_v[:, i, :], in_=wv_d.ap()[:, i, :])
            for i in range(NCI):
                nc.sync.dma_start(out=xc[i][:, W:2 * W],
                                  in_=xt_d.ap()[i * P:(i + 1) * P, W:2 * W])
            for j in (2, 3):
                dma_w_block(wh_q, wq_d, j)
                dma_w_block(wh_k, wk_d, j)
            for i in range(NCI):
                nc.sync.dma_start(out=xc[i][:, 2 * W:3 * W],
                                  in_=xt_d.ap()[i * P:(i + 1) * P, 2 * W:3 * W])
            nc.sync.dma_start(out=mask_sb, in_=mask_d.ap())
            nc.sync.dma_start(out=mask2_sb, in_=mask2_d.ap())
            for i in range(NCI):
                nc.sync.dma_start(out=xc[i][:, 3 * W:],
                                  in_=xt_d.ap()[i * P:(i + 1) * P, 3 * W:])
            nc.sync.dma_start(
                out=wo_sb, in_=wo_d.ap().rearrange("(c p) n -> p c n", p=P))

            # ---- constant init (off critical path, before first use) ----
            nc.vector.memset(ones_sb, 1.0)
            nc.vector.memset(kz[:, :, 1, :], 0.0)        # DoubleRow null slot
            nc.vector.memset(qT[:, CGP, :], 0.0)         # pad j-block
            for it in range(NTT):                        # ones columns early
                ones_view = vp[:, it, :].rearrange(
                    "p (h x) -> p h x", x=VS)[:, :, HD:VS]
                nc.vector.tensor_copy(
                    ones_view, ones_sb.rearrange("p (h x) -> p h x", x=1))

            # ---- phase B: projections (window-major) ----
            with tc.tile_pool(name="psum_b", bufs=4, space="PSUM") as psum_b:
                def emit_qk(j, w):
                    ws = slice(w * W, (w + 1) * W)
                    psq = psum_b.tile([P, W], f32, tag="psb")
                    for i in range(NCI):
                        mm(psq, wh_q[:, j, i, :], xc[i][:, ws],
                           start=(i == 0), stop=(i == NCI - 1))
                    nc.vector.tensor_scalar_add(qT[:, j, ws], psq,
                                                bq_sb[:, j:j + 1])
                    psk = psum_b.tile([P, W], f32, tag="psb")
                    for i in range(NCI):
                        mm(psk, wh_k[:, j, i, :], xc[i][:, ws],
                           start=(i == 0), stop=(i == NCI - 1))
                    nc.vector.tensor_scalar_add(kz[:, j, 0, ws], psk,
                                                bk_sb[:, j:j + 1])

                def emit_v(it):
                    psv = psum_b.tile([P, CG], f32, tag="psb")
                    for i in range(NCI):
                        mm(psv, xc[i][:, it * P:(it + 1) * P], wh_v[:, i, :],
                           start=(i == 0), stop=(i == NCI - 1))
                    v_view = vp[:, it, :].rearrange(
                        "p (h x) -> p h x", x=VS)[:, :, 0:HD]
                    nc.vector.tensor_copy(
                        v_view, psv.rearrange("p (h x) -> p h x", x=HD))

                for w in range(NW):
                    if w == 0:
                        emit_qk(0, 0)
                        emit_qk(1, 0)
                        for it in range(4):
                            emit_v(it)
                        emit_qk(2, 0)
                        emit_qk(3, 0)
                    else:
                        for it in range(4 * w, 4 * w + 4):
                            emit_v(it)
                        for j in range(CGP):
                            emit_qk(j, w)

            # ---- phase C: attention, software-pipelined ----
            with (
                tc.tile_pool(name="psum_mm", bufs=2, space="PSUM") as psum_mm,
                tc.tile_pool(name="psum_av", bufs=2, space="PSUM") as psum_av,
                tc.tile_pool(name="psum_o", bufs=2, space="PSUM") as psum_o,
            ):
                av_psum = {}

                def emit_qk_pair(w, h, x, npairs):
                    j, a = h // 2, h % 2
                    tq0 = w * W
                    i0, i1 = 2 * x, 2 * x + 1
                    vs0 = max(tq0, i0 * P)
                    n0 = tq0 + W - vs0
                    vs1 = max(tq0, i1 * P)
                    n1 = tq0 + W - vs1
                    rows = slice(a * HD, (a + 1) * HD)
                    ps_s = psum_mm.tile([P, 2, W], f32, tag="ps")
                    nc.tensor.matmul(
                        ps_s[:, 0, W - n0:],
                        kz[rows, j, :, i0 * P:(i0 + 1) * P],
                        qT[rows, j:j + 2, vs0:vs0 + n0],
                        start=True, stop=True, perf_mode=DR)
                    nc.tensor.matmul(
                        ps_s[:, 1, W - n1:],
                        kz[rows, j, :, i1 * P:(i1 + 1) * P],
                        qT[rows, j:j + 2, vs1:vs1 + n1],
                        start=True, stop=True, perf_mode=DR)
                    pt = ptpool.tile([P, 2, W], i16, tag="pt")
                    ptb = pt.bitcast(mybir.dt.bfloat16)
                    if h in SCH_HEADS:
                        nc.vector.tensor_scalar(
                            out=pt[:, :, W - n0:], in0=ps_s[:, :, W - n0:],
                            scalar1=ESC * LOG2E * 128.0,
                            scalar2=(127.0 - C_SCH) * 128.0,
                            op0=mybir.AluOpType.mult,
                            op1=mybir.AluOpType.add)
                    else:
                        nc.scalar.activation(ptb[:, :, W - n0:],
                                             ps_s[:, :, W - n0:],
                                             AF.Exp, scale=ESC)
                    if i0 * P >= tq0:  # diagonal pair
                        nc.vector.tensor_mul(
                            ptb[:, 0, W - n0:W - n0 + P],
                            ptb[:, 0, W - n0:W - n0 + P], mask_sb)
                        nc.vector.tensor_mul(
                            ptb[:, 1, W - n0:W - n0 + 2 * P],
                            ptb[:, 1, W - n0:W - n0 + 2 * P], mask2_sb)
                    return (w, h, x, npairs, ptb, n0, vs0)

                def emit_av(rec):
                    w, h, x, npairs, ptb, n0, vs0 = rec
                    tq0 = w * W
                    i0, i1 = 2 * x, 2 * x + 1
                    if x == 0:
                        av_psum[(w, h)] = psum_av.tile(
                            [VS, W], f32, tag="av", name=f"av_{w}_{h}")
                    ps_av = av_psum[(w, h)]
                    mm(ps_av[:, vs0 - tq0:], vp[:, i0, h * VS:(h + 1) * VS],
                       ptb[:, 0, W - n0:], start=(x == 0), stop=False)
                    mm(ps_av[:, vs0 - tq0:], vp[:, i1, h * VS:(h + 1) * VS],
                       ptb[:, 1, W - n0:], start=False, stop=(x == npairs - 1))
                    return x == npairs - 1

                def emit_norm(w, h):
                    j, a = h // 2, h % 2
                    tq0 = w * W
                    ps_av = av_psum.pop((w, h))
                    dn = dnpool.tile([1, W], f32, tag="dn")
                    nc.scalar.copy(dn, ps_av[HD:VS, :])
                    rb = rbpool.tile([HD, W], f32, tag="rb")
                    nc.gpsimd.partition_broadcast(rb, dn)
                    nc.vector.reciprocal_approx_fast(out=rb, in_=rb)
                    nc.vector.tensor_mul(
                        yT[a * HD:(a + 1) * HD, j, tq0:tq0 + W],
                        ps_av[0:HD, :], rb)

                def emit_outproj(w):
                    ws = slice(w * W, (w + 1) * W)
                    for m in range(C // P):
                        po = psum_o.tile([P, W], f32, tag="po")
                        for i in range(CGP):
                            mm(po, wo_sb[:, i, m * P:(m + 1) * P],
                               yT[:, i, ws],
                               start=(i == 0), stop=(i == CGP - 1))
                        ot = opool.tile([P, W], bf16, tag="ot")
                        nc.scalar.copy(ot, po)
                        nc.sync.dma_start(
                            out=out_d.ap()[m * P:(m + 1) * P, ws], in_=ot)

                pending = None
                outproj_due = None
                for w in range(NW):
                    npairs = 2 * (w + 1)
                    for h in range(HG):
                        for x in range(npairs):
                            rec = emit_qk_pair(w, h, x, npairs)
                            if pending is not None:
                                if emit_av(pending):
                                    pw, ph = pending[0], pending[1]
                                    emit_norm(pw, ph)
                                    if ph == HG - 1:
                                        outproj_due = pw
                                elif outproj_due is not None:
                                    emit_outproj(outproj_due)
                                    outproj_due = None
                            pending = rec
                # flush
                emit_av(pending)
                emit_norm(NW - 1, HG - 1)
                if outproj_due is not None:
                    emit_outproj(outproj_due)
                emit_outproj(NW - 1)

    nc.compile()
    return nc


def get_nc():
    global _cached_nc
    if _cached_nc is None:
        _cached_nc = _build()
    return _cached_nc


def make_in_maps(x, Wq, bq, Wk, bk, Wv, bv, Wo, bo):
    x = np.asarray(x, np.float32)
    mask = np.triu(np.ones((P, P), np.float32))
    mask2 = np.concatenate([np.zeros((P, P), np.float32), mask], axis=1)

    def wqk_layout(Wm, cs):
        m = np.asarray(Wm, np.float32)[:, cs].reshape(NCI, P, CGP, P)
        return np.ascontiguousarray(
            np.transpose(m, (1, 2, 0, 3)).reshape(P, CGP, NCI * P)
        ).astype(ml_dtypes.bfloat16)

    def wv_layout(Wm, cs):
        m = np.asarray(Wm, np.float32)[:, cs].reshape(NCI, P, CG)
        return np.ascontiguousarray(
            np.transpose(m, (1, 0, 2))).astype(ml_dtypes.bfloat16)

    in_maps = []
    for c in range(8):
        b, g = c // 2, c % 2
        cs = slice(g * CG, (g + 1) * CG)
        in_maps.append({
            "xt": np.ascontiguousarray(x[b].T.astype(ml_dtypes.bfloat16)),
            "wq": wqk_layout(Wq, cs),
            "wk": wqk_layout(Wk, cs),
            "wv": wv_layout(Wv, cs),
            "wo": np.ascontiguousarray(
                np.asarray(Wo, np.float32)[cs, :].astype(ml_dtypes.bfloat16)),
            "bq": np.ascontiguousarray(
                np.asarray(bq, np.float32)[cs].reshape(CGP, P).T),
            "bk": np.ascontiguousarray(
                np.asarray(bk, np.float32)[cs].reshape(CGP, P).T),
            "mask": mask,
            "mask2": mask2,
        })
    return in_maps


def combine(results, Wv, bv, Wo, bo):
    const = (np.asarray(bv, np.float32) @ np.asarray(Wo, np.float32)
             + np.asarray(bo, np.float32))
    out = np.empty((B, T, C), np.float32)
    for b in range(B):
        acc = (results[2 * b]["outp"].astype(np.float32)
               + results[2 * b + 1]["outp"].astype(np.float32))
        out[b] = acc.T + const[None, :]
    return out


def kernel(x, Wq, bq, Wk, bk, Wv, bv, Wo, bo):
    nc = get_nc()
    in_maps = make_in_maps(x, Wq, bq, Wk, bk, Wv, bv, Wo, bo)
    res = run_bass_kernel_spmd(nc, in_maps, core_ids=list(range(8)))
    return combine(res.results, Wv, bv, Wo, bo)


# revision 5
# speedup vs baseline: 1.0837x; 1.0837x over previous
"""Causal self-attention on 8 trn2 NeuronCores.

Sharding: core c -> (batch b = c//2, head-group g = c%2); each head-group is
8 heads = 512 channels.  Per core: q/k/v projections restricted to the
group's 512 columns, causal attention in the transposed orientation
S^T = [tk, tq] (softmax denominators come free from a ones-column in V),
partial output projection through the group's 512 rows of Wo.  Host sums
the two partials per batch and adds (bv @ Wo + bo).

The QK stationary operand is zero-padded to a full [128,128] footprint
(kz holds each head's k tile in its own 64 q-channel rows, zeros in the
other head's rows) — without this the PE activity monitor clock-gates the
tensor engine to 1.2 GHz.

Performance structure on top of that formulation:
- The attention inner loop is software-pipelined: the AV matmuls of tk-pair
  p are emitted after the QK matmuls + exp of pair p+1, so the exp of pair
  p overlaps the PE work of pair p+1 and the PE never waits on the exp.
- exp is split across engines per head: heads 3-7 use the scalar engine's
  ACT Exp; heads 0-2 use a Schraudolph fast-exp2 on the vector engine
  (tensor_scalar f32->int16 writing biased-exponent bits, consumed by the
  AV matmul through a bitcast-to-bf16 view).  Softmax normalization
  cancels most of the approximation error.
- Diagonal tk-tiles stream only their causally-needed columns on both the
  QK and AV sides; each diagonal tile takes one [128,128] triangular mask
  multiply, and the stale slice the strided exp touches is never read.
- PSUM->SBUF drain copies run on the scalar engine; output partials are
  written in bf16 to halve the drain DMA.
- DMA issue order is staged so the first projection matmuls start ~1us in.
"""

import numpy as np
import ml_dtypes

import concourse.bass as bass
import concourse.mybir as mybir
from concourse import bacc, tile
from concourse.bass_utils import run_bass_kernel_spmd

B, T, C, H = 4, 2048, 1024, 16
HD = C // H          # 64
G = 2                # head groups (cores per batch)
HG = H // G          # 8 heads per group
CG = C // G          # 512 channels per group
CGP = CG // 128      # 4 c_out tiles per group
P = 128
W = 512              # free-dim window (one PSUM bank of f32)
NW = T // W          # 4 windows
NTT = T // P         # 16 t tiles
NCI = C // P         # 8 c_in chunks
VS = HD + 1          # 65: v plus ones column

ESC = 1.0 / float(np.sqrt(HD))
LOG2E = 1.4426950408889634
C_SCH = 0.045        # Schraudolph bias constant (tuned by simulation)
SCH_HEADS = frozenset({0, 1, 2})   # heads whose exp runs on the DVE

_cached_nc = None


def _build():
    f32 = mybir.dt.float32
    bf16 = mybir.dt.bfloat16
    i16 = mybir.dt.int16
    AF = mybir.ActivationFunctionType
    nc = bacc.Bacc("TRN2", target_bir_lowering=False, debug=False, num_devices=8)

    xt_d = nc.dram_tensor("xt", [C, T], bf16, kind="ExternalInput")
    wq_d = nc.dram_tensor("wq", [P, CGP, NCI * P], bf16, kind="ExternalInput")
    wk_d = nc.dram_tensor("wk", [P, CGP, NCI * P], bf16, kind="ExternalInput")
    wv_d = nc.dram_tensor("wv", [P, NCI, CG], bf16, kind="ExternalInput")
    wo_d = nc.dram_tensor("wo", [CG, C], bf16, kind="ExternalInput")
    bq_d = nc.dram_tensor("bq", [P, CGP], f32, kind="ExternalInput")
    bk_d = nc.dram_tensor("bk", [P, CGP], f32, kind="ExternalInput")
    mask_d = nc.dram_tensor("mask", [P, P], f32, kind="ExternalInput")
    out_d = nc.dram_tensor("outp", [C, T], bf16, kind="ExternalOutput")

    mm = lambda out, lhsT, rhs, start, stop: nc.tensor.matmul(
        out, lhsT, rhs, start=start, stop=stop)

    with tile.TileContext(nc) as tc:
        with (
            tc.tile_pool(name="pers", bufs=1) as pers,
            tc.tile_pool(name="wchunk", bufs=1) as wpool,
            tc.tile_pool(name="xchunk", bufs=NCI) as xpool,
            tc.tile_pool(name="attn", bufs=1) as attn,
            tc.tile_pool(name="pt", bufs=4) as ptpool,
            tc.tile_pool(name="dn", bufs=4) as dnpool,
            tc.tile_pool(name="rb", bufs=4) as rbpool,
            tc.tile_pool(name="osb", bufs=3) as opool,
        ):
            qT = pers.tile([P, CGP, T], bf16)        # q^T: [c_out, t]
            # kz[:, j, a, tk]: head h=2j+a k-tile in its own 64 rows, 0 else
            kz = pers.tile([P, CGP, 2, T], bf16)
            vp = pers.tile([P, NTT, HG * VS], bf16)  # v rows + ones col/head
            wo_sb = pers.tile([P, CGP, C], bf16)
            mask_sb = pers.tile([P, P], f32)
            bq_sb = pers.tile([P, CGP], f32)
            bk_sb = pers.tile([P, CGP], f32)
            ones_sb = pers.tile([P, HG], f32)
            yT = attn.tile([P, CGP, T], bf16)

            # ---- DMA issue order tuned for startup ----
            def dma_w_block(dst, src_d, j):
                nc.sync.dma_start(out=dst[:, j, :, :],
                                  in_=src_d.ap()[:, j, :].rearrange(
                                      "p (c n) -> p c n", n=P))

            wh_q = wpool.tile([P, CGP, NCI, P], bf16)
            wh_k = wpool.tile([P, CGP, NCI, P], bf16)
            wh_v = wpool.tile([P, NCI, CG], bf16)
            xc = [xpool.tile([P, T], bf16, tag="xc", name=f"xc{i}")
                  for i in range(NCI)]

            dma_w_block(wh_q, wq_d, 0)
            dma_w_block(wh_k, wk_d, 0)
            nc.sync.dma_start(out=bq_sb, in_=bq_d.ap())
            nc.sync.dma_start(out=bk_sb, in_=bk_d.ap())
            for i in range(NCI):   # quarter 0 of all chunks
                nc.sync.dma_start(out=xc[i][:, 0:W],
                                  in_=xt_d.ap()[i * P:(i + 1) * P, 0:W])
            dma_w_block(wh_q, wq_d, 1)
            dma_w_block(wh_k, wk_d, 1)
            for i in range(NCI):   # wv per chunk
                nc.sync.dma_start(out=wh_v[:, i, :], in_=wv_d.ap()[:, i, :])
            for i in range(NCI):
                nc.sync.dma_start(out=xc[i][:, W:2 * W],
                                  in_=xt_d.ap()[i * P:(i + 1) * P, W:2 * W])
            for j in (2, 3):
                dma_w_block(wh_q, wq_d, j)
                dma_w_block(wh_k, wk_d, j)
            for i in range(NCI):
                nc.sync.dma_start(out=xc[i][:, 2 * W:3 * W],
                                  in_=xt_d.ap()[i * P:(i + 1) * P, 2 * W:3 * W])
            nc.sync.dma_start(out=mask_sb, in_=mask_d.ap())
            for i in range(NCI):
                nc.sync.dma_start(out=xc[i][:, 3 * W:],
                                  in_=xt_d.ap()[i * P:(i + 1) * P, 3 * W:])
            nc.sync.dma_start(
                out=wo_sb, in_=wo_d.ap().rearrange("(c p) n -> p c n", p=P))

            # ---- constant init (off critical path, before first use) ----
            nc.vector.memset(ones_sb, 1.0)
            nc.vector.memset(kz[0:HD, :, 1, :], 0.0)
            nc.vector.memset(kz[HD:P, :, 0, :], 0.0)
            for it in range(NTT):                    # ones columns early
                ones_view = vp[:, it, :].rearrange(
                    "p (h x) -> p h x", x=VS)[:, :, HD:VS]
                nc.vector.tensor_copy(
                    ones_view, ones_sb.rearrange("p (h x) -> p h x", x=1))

            # ---- phase B: projections (window-major) ----
            with tc.tile_pool(name="psum_b", bufs=4, space="PSUM") as psum_b:
                def emit_qk(j, w):
                    ws = slice(w * W, (w + 1) * W)
                    psq = psum_b.tile([P, W], f32, tag="psb")
                    for i in range(NCI):
                        mm(psq, wh_q[:, j, i, :], xc[i][:, ws],
                           start=(i == 0), stop=(i == NCI - 1))
                    nc.vector.tensor_scalar_add(qT[:, j, ws], psq,
                                                bq_sb[:, j:j + 1])
                    psk = psum_b.tile([P, W], f32, tag="psb")
                    for i in range(NCI):
                        mm(psk, wh_k[:, j, i, :], xc[i][:, ws],
                           start=(i == 0), stop=(i == NCI - 1))
                    for a in range(2):
                        rows = slice(a * HD, (a + 1) * HD)
                        nc.vector.tensor_scalar_add(kz[rows, j, a, ws],
                                                    psk[rows, :],
                                                    bk_sb[rows, j:j + 1])

                def emit_v(it):
                    psv = psum_b.tile([P, CG], f32, tag="psb")
                    for i in range(NCI):
                        mm(psv, xc[i][:, it * P:(it + 1) * P], wh_v[:, i, :],
                           start=(i == 0), stop=(i == NCI - 1))
                    v_view = vp[:, it, :].rearrange(
                        "p (h x) -> p h x", x=VS)[:, :, 0:HD]
                    nc.vector.tensor_copy(
                        v_view, psv.rearrange("p (h x) -> p h x", x=HD))

                for w in range(NW):
                    if w == 0:
                        emit_qk(0, 0)
                        emit_qk(1, 0)
                        for it in range(4):
                            emit_v(it)
                        emit_qk(2, 0)
                        emit_qk(3, 0)
                    else:
                        for it in range(4 * w, 4 * w + 4):
                            emit_v(it)
                        for j in range(CGP):
                            emit_qk(j, w)

            # ---- phase C: attention, software-pipelined ----
            with (
                tc.tile_pool(name="psum_mm", bufs=2, space="PSUM") as psum_mm,
                tc.tile_pool(name="psum_av", bufs=2, space="PSUM") as psum_av,
                tc.tile_pool(name="psum_o", bufs=2, space="PSUM") as psum_o,
            ):
                av_psum = {}

                def emit_qk_pair(w, h, x, npairs):
                    j, a = h // 2, h % 2
                    tq0 = w * W
                    i0, i1 = 2 * x, 2 * x + 1
                    vs0 = max(tq0, i0 * P)
                    n0 = tq0 + W - vs0
                    vs1 = max(tq0, i1 * P)
                    n1 = tq0 + W - vs1
                    ps_s = psum_mm.tile([P, 2, W], f32, tag="ps")
                    mm(ps_s[:, 0, W - n0:], kz[:, j, a, i0 * P:(i0 + 1) * P],
                       qT[:, j, vs0:vs0 + n0], start=True, stop=True)
                    mm(ps_s[:, 1, W - n1:], kz[:, j, a, i1 * P:(i1 + 1) * P],
                       qT[:, j, vs1:vs1 + n1], start=True, stop=True)
                    pt = ptpool.tile([P, 2, W], i16, tag="pt")
                    ptb = pt.bitcast(mybir.dt.bfloat16)
                    if h in SCH_HEADS:
                        nc.vector.tensor_scalar(
                            out=pt[:, :, W - n0:], in0=ps_s[:, :, W - n0:],
                            scalar1=ESC * LOG2E * 128.0,
                            scalar2=(127.0 - C_SCH) * 128.0,
                            op0=mybir.AluOpType.mult,
                            op1=mybir.AluOpType.add)
                    else:
                        nc.scalar.activation(ptb[:, :, W - n0:],
                                             ps_s[:, :, W - n0:],
                                             AF.Exp, scale=ESC)
                    if i0 * P >= tq0:  # diagonal tiles: triangular masks
                        nc.vector.tensor_mul(
                            ptb[:, 0, W - n0:W - n0 + P],
                            ptb[:, 0, W - n0:W - n0 + P], mask_sb)
                        nc.vector.tensor_mul(
                            ptb[:, 1, W - n1:W - n1 + P],
                            ptb[:, 1, W - n1:W - n1 + P], mask_sb)
                    return (w, h, x, npairs, ptb, n0, vs0, n1, vs1)

                def emit_av(rec):
                    w, h, x, npairs, ptb, n0, vs0, n1, vs1 = rec
                    tq0 = w * W
                    i0, i1 = 2 * x, 2 * x + 1
                    if x == 0:
                        av_psum[(w, h)] = psum_av.tile(
                            [VS, W], f32, tag="av", name=f"av_{w}_{h}")
                    ps_av = av_psum[(w, h)]
                    mm(ps_av[:, vs0 - tq0:], vp[:, i0, h * VS:(h + 1) * VS],
                       ptb[:, 0, W - n0:], start=(x == 0), stop=False)
                    mm(ps_av[:, vs1 - tq0:], vp[:, i1, h * VS:(h + 1) * VS],
                       ptb[:, 1, W - n1:], start=False, stop=(x == npairs - 1))
                    return x == npairs - 1

                def emit_norm(w, h):
                    j, a = h // 2, h % 2
                    tq0 = w * W
                    ps_av = av_psum.pop((w, h))
                    dn = dnpool.tile([1, W], f32, tag="dn")
                    nc.scalar.copy(dn, ps_av[HD:VS, :])
                    rb = rbpool.tile([HD, W], f32, tag="rb")
                    nc.gpsimd.partition_broadcast(rb, dn)
                    nc.vector.reciprocal_approx_fast(out=rb, in_=rb)
                    nc.vector.tensor_mul(
                        yT[a * HD:(a + 1) * HD, j, tq0:tq0 + W],
                        ps_av[0:HD, :], rb)

                def emit_outproj(w):
                    ws = slice(w * W, (w + 1) * W)
                    for m in range(C // P):
                        po = psum_o.tile([P, W], f32, tag="po")
                        for i in range(CGP):
                            mm(po, wo_sb[:, i, m * P:(m + 1) * P],
                               yT[:, i, ws],
                               start=(i == 0), stop=(i == CGP - 1))
                        ot = opool.tile([P, W], bf16, tag="ot")
                        nc.scalar.copy(ot, po)
                        nc.sync.dma_start(
                            out=out_d.ap()[m * P:(m + 1) * P, ws], in_=ot)

                pending = None
                outproj_due = None
                for w in range(NW):
                    npairs = 2 * (w + 1)
                    for h in range(HG):
                        for x in range(npairs):
                            rec = emit_qk_pair(w, h, x, npairs)
                            if pending is not None:
                                if emit_av(pending):
                                    pw, ph = pending[0], pending[1]
                                    emit_norm(pw, ph)
                                    if ph == HG - 1:
                                        outproj_due = pw
                                elif outproj_due is not None:
                                    emit_outproj(outproj_due)
                                    outproj_due = None
                            pending = rec
                # flush
                emit_av(pending)
                emit_norm(NW - 1, HG - 1)
                if outproj_due is not None:
                    emit_outproj(outproj_due)
                emit_outproj(NW - 1)

    nc.compile()
    return nc


def get_nc():
    global _cached_nc
    if _cached_nc is None:
        _cached_nc = _build()
    return _cached_nc


def make_in_maps(x, Wq, bq, Wk, bk, Wv, bv, Wo, bo):
    x = np.asarray(x, np.float32)
    mask = np.triu(np.ones((P, P), np.float32))

    def wqk_layout(Wm, cs):
        m = np.asarray(Wm, np.float32)[:, cs].reshape(NCI, P, CGP, P)
        return np.ascontiguousarray(
            np.transpose(m, (1, 2, 0, 3)).reshape(P, CGP, NCI * P)
        ).astype(ml_dtypes.bfloat16)

    def wv_layout(Wm, cs):
        m = np.asarray(Wm, np.float32)[:, cs].reshape(NCI, P, CG)
        return np.ascontiguousarray(
            np.transpose(m, (1, 0, 2))).astype(ml_dtypes.bfloat16)

    in_maps = []
    for c in range(8):
        b, g = c // 2, c % 2
        cs = slice(g * CG, (g + 1) * CG)
        in_maps.append({
            "xt": np.ascontiguousarray(x[b].T.astype(ml_dtypes.bfloat16)),
            "wq": wqk_layout(Wq, cs),
            "wk": wqk_layout(Wk, cs),
            "wv": wv_layout(Wv, cs),
            "wo": np.ascontiguousarray(
                np.asarray(Wo, np.float32)[cs, :].astype(ml_dtypes.bfloat16)),
            "bq": np.ascontiguousarray(
                np.asarray(bq, np.float32)[cs].reshape(CGP, P).T),
            "bk": np.ascontiguousarray(
                np.asarray(bk, np.float32)[cs].reshape(CGP, P).T),
            "mask": mask,
        })
    return in_maps


def combine(results, Wv, bv, Wo, bo):
    const = (np.asarray(bv, np.float32) @ np.asarray(Wo, np.float32)
             + np.asarray(bo, np.float32))
    out = np.empty((B, T, C), np.float32)
    for b in range(B):
        acc = (results[2 * b]["outp"].astype(np.float32)
               + results[2 * b + 1]["outp"].astype(np.float32))
        out[b] = acc.T + const[None, :]
    return out


def kernel(x, Wq, bq, Wk, bk, Wv, bv, Wo, bo):
    nc = get_nc()
    in_maps = make_in_maps(x, Wq, bq, Wk, bk, Wv, bv, Wo, bo)
    res = run_bass_kernel_spmd(nc, in_maps, core_ids=list(range(8)))
    return combine(res.results, Wv, bv, Wo, bo)


# revision 9
# speedup vs baseline: 1.1676x; 1.0774x over previous
"""Causal self-attention on 8 trn2 NeuronCores.

Sharding: core c -> (batch b = c//2, head-group g = c%2); each head-group is
8 heads = 512 channels.  Per core: q/k/v projections restricted to the
group's 512 columns, causal attention in the transposed orientation
S^T = [tk, tq] (softmax denominators come free from a ones-column in V),
partial output projection through the group's 512 rows of Wo.  Host sums
the two partials per batch and adds (bv @ Wo + bo).

The QK stationary operand is zero-padded to a full [128,128] footprint
(kz holds each head's k tile in its own 64 q-channel rows, zeros in the
other head's rows) — without this the PE activity monitor clock-gates the
tensor engine to 1.2 GHz.

Performance structure on top of that formulation:
- The attention inner loop is software-pipelined: the AV matmuls of tk-pair
  p are emitted after the QK matmuls + exp of pair p+1, so the exp of pair
  p overlaps the PE work of pair p+1 and the PE never waits on the exp.
- exp is split across engines per head: heads 3-7 use the scalar engine's
  ACT Exp; heads 0-2 use a Schraudolph fast-exp2 on the vector engine
  (tensor_scalar f32->int16 writing biased-exponent bits, consumed by the
  AV matmul through a bitcast-to-bf16 view).  Softmax normalization
  cancels most of the approximation error.
- Diagonal tk-tiles stream only their causally-needed columns on both the
  QK and AV sides; each diagonal tile takes one [128,128] triangular mask
  multiply, and the stale slice the strided exp touches is never read.
- PSUM->SBUF drain copies run on the scalar engine; output partials are
  written in bf16 to halve the drain DMA.
- DMA issue order is staged so the first projection matmuls start ~1us in.
"""

import numpy as np
import ml_dtypes

import concourse.bass as bass
import concourse.mybir as mybir
from concourse import bacc, tile
from concourse.bass_utils import run_bass_kernel_spmd

B, T, C, H = 4, 2048, 1024, 16
HD = C // H          # 64
G = 2                # head groups (cores per batch)
HG = H // G          # 8 heads per group
CG = C // G          # 512 channels per group
CGP = CG // 128      # 4 c_out tiles per group
P = 128
W = 512              # free-dim window (one PSUM bank of f32)
NW = T // W          # 4 windows
NTT = T // P         # 16 t tiles
NCI = C // P         # 8 c_in chunks
VS = HD + 1          # 65: v plus ones column

ESC = 1.0 / float(np.sqrt(HD))
LOG2E = 1.4426950408889634
C_SCH = 0.045        # Schraudolph bias constant (tuned by simulation)
SCH_MOD = 3          # every SCH_MOD-th pair's exp runs on the DVE

_cached_nc = None


def _build():
    f32 = mybir.dt.float32
    bf16 = mybir.dt.bfloat16
    i16 = mybir.dt.int16
    AF = mybir.ActivationFunctionType
    nc = bacc.Bacc("TRN2", target_bir_lowering=False, debug=False, num_devices=8)

    xt_d = nc.dram_tensor("xt", [C, T], bf16, kind="ExternalInput")
    wq_d = nc.dram_tensor("wq", [P, CGP, NCI * P], bf16, kind="ExternalInput")
    wk_d = nc.dram_tensor("wk", [P, CGP, NCI * P], bf16, kind="ExternalInput")
    wv_d = nc.dram_tensor("wv", [P, NCI, CG], bf16, kind="ExternalInput")
    wo_d = nc.dram_tensor("wo", [CG, C], bf16, kind="ExternalInput")
    bq_d = nc.dram_tensor("bq", [P, CGP], f32, kind="ExternalInput")
    bk_d = nc.dram_tensor("bk", [P, CGP], f32, kind="ExternalInput")
    mask_d = nc.dram_tensor("mask", [P, P], f32, kind="ExternalInput")
    out_d = nc.dram_tensor("outp", [C, T], bf16, kind="ExternalOutput")

    mm = lambda out, lhsT, rhs, start, stop: nc.tensor.matmul(
        out, lhsT, rhs, start=start, stop=stop)

    with tile.TileContext(nc) as tc:
        with (
            tc.tile_pool(name="pers", bufs=1) as pers,
            tc.tile_pool(name="wchunk", bufs=1) as wpool,
            tc.tile_pool(name="xchunk", bufs=NCI) as xpool,
            tc.tile_pool(name="attn", bufs=1) as attn,
            tc.tile_pool(name="pt", bufs=4) as ptpool,
            tc.tile_pool(name="dn", bufs=4) as dnpool,
            tc.tile_pool(name="rb", bufs=4) as rbpool,
            tc.tile_pool(name="osb", bufs=3) as opool,
        ):
            qT = pers.tile([P, CGP, T], bf16)        # q^T: [c_out, t]
            # kz[:, j, a, tk]: head h=2j+a k-tile in its own 64 rows, 0 else
            kz = pers.tile([P, CGP, 2, T], bf16)
            vp = pers.tile([P, NTT, HG * VS], bf16)  # v rows + ones col/head
            wo_sb = pers.tile([P, CGP, C], bf16)
            mask_sb = pers.tile([P, P], f32)
            bq_sb = pers.tile([P, CGP], f32)
            bk_sb = pers.tile([P, CGP], f32)
            ones_sb = pers.tile([P, HG], f32)
            yT = attn.tile([P, CGP, T], bf16)

            # ---- DMA issue order tuned for startup ----
            def dma_w_block(dst, src_d, j):
                nc.sync.dma_start(out=dst[:, j, :, :],
                                  in_=src_d.ap()[:, j, :].rearrange(
                                      "p (c n) -> p c n", n=P))

            wh_q = wpool.tile([P, CGP, NCI, P], bf16)
            wh_k = wpool.tile([P, CGP, NCI, P], bf16)
            wh_v = wpool.tile([P, NCI, CG], bf16)
            xc = [xpool.tile([P, T], bf16, tag="xc", name=f"xc{i}")
                  for i in range(NCI)]

            dma_w_block(wh_q, wq_d, 0)
            dma_w_block(wh_k, wk_d, 0)
            nc.sync.dma_start(out=bq_sb, in_=bq_d.ap())
            nc.sync.dma_start(out=bk_sb, in_=bk_d.ap())
            for i in range(NCI):   # quarter 0 of all chunks
                nc.sync.dma_start(out=xc[i][:, 0:W],
                                  in_=xt_d.ap()[i * P:(i + 1) * P, 0:W])
            dma_w_block(wh_q, wq_d, 1)
            dma_w_block(wh_k, wk_d, 1)
            for i in range(NCI):   # wv per chunk
                nc.sync.dma_start(out=wh_v[:, i, :], in_=wv_d.ap()[:, i, :])
            for i in range(NCI):
                nc.sync.dma_start(out=xc[i][:, W:2 * W],
                                  in_=xt_d.ap()[i * P:(i + 1) * P, W:2 * W])
            for j in (2, 3):
                dma_w_block(wh_q, wq_d, j)
                dma_w_block(wh_k, wk_d, j)
            for i in range(NCI):
                nc.sync.dma_start(out=xc[i][:, 2 * W:3 * W],
                                  in_=xt_d.ap()[i * P:(i + 1) * P, 2 * W:3 * W])
            nc.sync.dma_start(out=mask_sb, in_=mask_d.ap())
            for i in range(NCI):
                nc.sync.dma_start(out=xc[i][:, 3 * W:],
                                  in_=xt_d.ap()[i * P:(i + 1) * P, 3 * W:])
            nc.sync.dma_start(
                out=wo_sb, in_=wo_d.ap().rearrange("(c p) n -> p c n", p=P))

            # ---- constant init on gpsimd: runs parallel to phase B, done
            # well before the first attention read ----
            nc.gpsimd.memset(ones_sb, 1.0)
            nc.gpsimd.memset(kz[0:HD, :, 1, :], 0.0)
            nc.gpsimd.memset(kz[HD:P, :, 0, :], 0.0)
            for it in range(NTT):                    # ones columns early
                ones_view = vp[:, it, :].rearrange(
                    "p (h x) -> p h x", x=VS)[:, :, HD:VS]
                nc.gpsimd.tensor_copy(
                    ones_view, ones_sb.rearrange("p (h x) -> p h x", x=1))

            # ---- phase B: projections (window-major) ----
            with tc.tile_pool(name="psum_b", bufs=4, space="PSUM") as psum_b:
                def emit_qk(j, w):
                    ws = slice(w * W, (w + 1) * W)
                    psq = psum_b.tile([P, W], f32, tag="psb")
                    for i in range(NCI):
                        mm(psq, wh_q[:, j, i, :], xc[i][:, ws],
                           start=(i == 0), stop=(i == NCI - 1))
                    nc.vector.tensor_scalar_add(qT[:, j, ws], psq,
                                                bq_sb[:, j:j + 1])
                    psk = psum_b.tile([P, W], f32, tag="psb")
                    for i in range(NCI):
                        mm(psk, wh_k[:, j, i, :], xc[i][:, ws],
                           start=(i == 0), stop=(i == NCI - 1))
                    for a in range(2):
                        rows = slice(a * HD, (a + 1) * HD)
                        nc.vector.tensor_scalar_add(kz[rows, j, a, ws],
                                                    psk[rows, :],
                                                    bk_sb[rows, j:j + 1])

                def emit_v(it):
                    psv = psum_b.tile([P, CG], f32, tag="psb")
                    for i in range(NCI):
                        mm(psv, xc[i][:, it * P:(it + 1) * P], wh_v[:, i, :],
                           start=(i == 0), stop=(i == NCI - 1))
                    v_view = vp[:, it, :].rearrange(
                        "p (h x) -> p h x", x=VS)[:, :, 0:HD]
                    nc.vector.tensor_copy(
                        v_view, psv.rearrange("p (h x) -> p h x", x=HD))

                for w in range(NW):
                    if w == 0:
                        emit_qk(0, 0)
                        emit_qk(1, 0)
                        for it in range(4):
                            emit_v(it)
                        emit_qk(2, 0)
                        emit_qk(3, 0)
                    else:
                        for it in range(4 * w, 4 * w + 4):
                            emit_v(it)
                        for j in range(CGP):
                            emit_qk(j, w)

            # ---- phase C: attention, software-pipelined ----
            with (
                tc.tile_pool(name="psum_mm", bufs=2, space="PSUM") as psum_mm,
                tc.tile_pool(name="psum_av", bufs=2, space="PSUM") as psum_av,
                tc.tile_pool(name="psum_o", bufs=2, space="PSUM") as psum_o,
            ):
                av_psum = {}
                pair_seq = [0]  # global pair counter for exp-engine choice

                def emit_qk_pair(w, h, x, npairs):
                    j, a = h // 2, h % 2
                    tq0 = w * W
                    i0, i1 = 2 * x, 2 * x + 1
                    vs0 = max(tq0, i0 * P)
                    n0 = tq0 + W - vs0
                    vs1 = max(tq0, i1 * P)
                    n1 = tq0 + W - vs1
                    ps_s = psum_mm.tile([P, 2, W], f32, tag="ps")
                    mm(ps_s[:, 0, W - n0:], kz[:, j, a, i0 * P:(i0 + 1) * P],
                       qT[:, j, vs0:vs0 + n0], start=True, stop=True)
                    mm(ps_s[:, 1, W - n1:], kz[:, j, a, i1 * P:(i1 + 1) * P],
                       qT[:, j, vs1:vs1 + n1], start=True, stop=True)
                    pt = ptpool.tile([P, 2, W], i16, tag="pt")
                    ptb = pt.bitcast(mybir.dt.bfloat16)
                    use_dve = pair_seq[0] % SCH_MOD == SCH_MOD - 1
                    pair_seq[0] += 1
                    if use_dve:
                        nc.vector.tensor_scalar(
                            out=pt[:, :, W - n0:], in0=ps_s[:, :, W - n0:],
                            scalar1=ESC * LOG2E * 128.0,
                            scalar2=(127.0 - C_SCH) * 128.0,
                            op0=mybir.AluOpType.mult,
                            op1=mybir.AluOpType.add)
                    else:
                        nc.scalar.activation(ptb[:, :, W - n0:],
                                             ps_s[:, :, W - n0:],
                                             AF.Exp, scale=ESC)
                    if i0 * P >= tq0:  # diagonal tiles: triangular masks
                        nc.vector.tensor_mul(
                            ptb[:, 0, W - n0:W - n0 + P],
                            ptb[:, 0, W - n0:W - n0 + P], mask_sb)
                        nc.vector.tensor_mul(
                            ptb[:, 1, W - n1:W - n1 + P],
                            ptb[:, 1, W - n1:W - n1 + P], mask_sb)
                    return (w, h, x, npairs, ptb, n0, vs0, n1, vs1)

                def emit_av(rec):
                    w, h, x, npairs, ptb, n0, vs0, n1, vs1 = rec
                    tq0 = w * W
                    i0, i1 = 2 * x, 2 * x + 1
                    if x == 0:
                        av_psum[(w, h)] = psum_av.tile(
                            [VS, W], f32, tag="av", name=f"av_{w}_{h}")
                    ps_av = av_psum[(w, h)]
                    mm(ps_av[:, vs0 - tq0:], vp[:, i0, h * VS:(h + 1) * VS],
                       ptb[:, 0, W - n0:], start=(x == 0), stop=False)
                    mm(ps_av[:, vs1 - tq0:], vp[:, i1, h * VS:(h + 1) * VS],
                       ptb[:, 1, W - n1:], start=False, stop=(x == npairs - 1))
                    return x == npairs - 1

                def emit_norm(w, h):
                    j, a = h // 2, h % 2
                    tq0 = w * W
                    ps_av = av_psum.pop((w, h))
                    dn = dnpool.tile([1, W], f32, tag="dn")
                    nc.scalar.copy(dn, ps_av[HD:VS, :])
                    rb = rbpool.tile([HD, W], f32, tag="rb")
                    nc.gpsimd.partition_broadcast(rb, dn)
                    nc.vector.reciprocal_approx_fast(out=rb, in_=rb)
                    nc.vector.tensor_mul(
                        yT[a * HD:(a + 1) * HD, j, tq0:tq0 + W],
                        ps_av[0:HD, :], rb)

                def emit_outproj(w):
                    ws = slice(w * W, (w + 1) * W)
                    for m in range(C // P):
                        po = psum_o.tile([P, W], f32, tag="po")
                        for i in range(CGP):
                            mm(po, wo_sb[:, i, m * P:(m + 1) * P],
                               yT[:, i, ws],
                               start=(i == 0), stop=(i == CGP - 1))
                        ot = opool.tile([P, W], bf16, tag="ot")
                        nc.scalar.copy(ot, po)
                        nc.sync.dma_start(
                            out=out_d.ap()[m * P:(m + 1) * P, ws], in_=ot)

                pending = None
                outproj_due = None
                for w in range(NW):
                    npairs = 2 * (w + 1)
                    for h in range(HG):
                        for x in range(npairs):
                            rec = emit_qk_pair(w, h, x, npairs)
                            if pending is not None:
                                if emit_av(pending):
                                    pw, ph = pending[0], pending[1]
                                    emit_norm(pw, ph)
                                    if ph == HG - 1:
                                        outproj_due = pw
                                elif outproj_due is not None:
                                    emit_outproj(outproj_due)
                                    outproj_due = None
                            pending = rec
                # flush
                emit_av(pending)
                emit_norm(NW - 1, HG - 1)
                if outproj_due is not None:
                    emit_outproj(outproj_due)
                emit_outproj(NW - 1)

    nc.compile()
    return nc


def get_nc():
    global _cached_nc
    if _cached_nc is None:
        _cached_nc = _build()
    return _cached_nc


def make_in_maps(x, Wq, bq, Wk, bk, Wv, bv, Wo, bo):
    x = np.asarray(x, np.float32)
    mask = np.triu(np.ones((P, P), np.float32))

    def wqk_layout(Wm, cs):
        m = np.asarray(Wm, np.float32)[:, cs].reshape(NCI, P, CGP, P)
        return np.ascontiguousarray(
            np.transpose(m, (1, 2, 0, 3)).reshape(P, CGP, NCI * P)
        ).astype(ml_dtypes.bfloat16)

    def wv_layout(Wm, cs):
        m = np.asarray(Wm, np.float32)[:, cs].reshape(NCI, P, CG)
        return np.ascontiguousarray(
            np.transpose(m, (1, 0, 2))).astype(ml_dtypes.bfloat16)

    in_maps = []
    for c in range(8):
        b, g = c // 2, c % 2
        cs = slice(g * CG, (g + 1) * CG)
        in_maps.append({
            "xt": np.ascontiguousarray(x[b].T.astype(ml_dtypes.bfloat16)),
            "wq": wqk_layout(Wq, cs),
            "wk": wqk_layout(Wk, cs),
            "wv": wv_layout(Wv, cs),
            "wo": np.ascontiguousarray(
                np.asarray(Wo, np.float32)[cs, :].astype(ml_dtypes.bfloat16)),
            "bq": np.ascontiguousarray(
                np.asarray(bq, np.float32)[cs].reshape(CGP, P).T),
            "bk": np.ascontiguousarray(
                np.asarray(bk, np.float32)[cs].reshape(CGP, P).T),
            "mask": mask,
        })
    return in_maps


def combine(results, Wv, bv, Wo, bo):
    const = (np.asarray(bv, np.float32) @ np.asarray(Wo, np.float32)
             + np.asarray(bo, np.float32))
    out = np.empty((B, T, C), np.float32)
    for b in range(B):
        acc = (results[2 * b]["outp"].astype(np.float32)
               + results[2 * b + 1]["outp"].astype(np.float32))
        out[b] = acc.T + const[None, :]
    return out


def kernel(x, Wq, bq, Wk, bk, Wv, bv, Wo, bo):
    nc = get_nc()
    in_maps = make_in_maps(x, Wq, bq, Wk, bk, Wv, bv, Wo, bo)
    res = run_bass_kernel_spmd(nc, in_maps, core_ids=list(range(8)))
    return combine(res.results, Wv, bv, Wo, bo)
